# revision 3
# baseline (speedup 1.0000x reference)
"""DGCN diffusion-graph-conv kernel for 8 Trainium2 NeuronCores.

Math (per the reference):
    support S = D^-1/2 (adj+I)^T D^-1/2  with D = diag(rowsum(adj+I))
    x_m = T_m(S) x0  (Chebyshev recurrence, K=3 -> m=0..3)
    out = sum_m x_m @ W_m + bias

Implementation strategy (data-parallel over batch, 4 batches/core):
    Rewrite out = sum_m T_m(S) (x0 @ W_m) and fold the Chebyshev
    coefficients into the weights:
        V0 = W0 - W2, V1 = W1 - 3*W3, V2 = 2*W2, V3 = 4*W3
        U_m = x0 @ V_m   (projection; contracts feature dim d)
        out = U0 + S*(U1 + S*(U2 + S*U3))   (Horner; contracts node dim n)
    This needs only one transpose (x0 -> x0^T for the projection's
    stationary operand), done on the tensor engine.  All matmuls run in
    fp32r (fp22 multiply / fp32 accumulate) at full PE rate.
"""

import numpy as np

import concourse.bacc as bacc
import concourse.tile as tile
import concourse.mybir as mybir
from concourse.bass_utils import run_bass_kernel_spmd

F32 = mybir.dt.float32
F32R = mybir.dt.float32r
AX = mybir.AxisListType
ALU = mybir.AluOpType

N_CORES = 8
B, N, D = 32, 512, 768
BL = B // N_CORES          # local batches per core = 4
BN = BL * N                # local rows = 2048
NT = BN // 128             # 16 row tiles
DT = D // 128              # 6 feature tiles
JT = N // 128              # 4 node tiles
WE = 256                   # output-column block width
EB = D // WE               # 3 column blocks


def _build_program():
    nc = bacc.Bacc("TRN2", target_bir_lowering=False, debug=False,
                   num_devices=N_CORES)
    inp_d = nc.dram_tensor("inp", [BN, D], F32, kind="ExternalInput").ap()
    adj_d = nc.dram_tensor("adj", [N, N], F32, kind="ExternalInput").ap()
    wts_d = nc.dram_tensor("wts", [D * 4, D], F32, kind="ExternalInput").ap()
    bias_d = nc.dram_tensor("bias", [D], F32, kind="ExternalInput").ap()
    eye_d = nc.dram_tensor("eye", [N, N], F32, kind="ExternalInput").ap()
    out_d = nc.dram_tensor("out", [BN, D], F32, kind="ExternalOutput").ap()
    dscr = nc.dram_tensor("dscr", [N], F32)

    # weights viewed as [m, d, e] (reference row index is d*4+m)
    wts_v = wts_d.rearrange("(d m) e -> m d e", m=4)

    with tile.TileContext(nc) as tc:
        with (
            tc.tile_pool(name="const", bufs=1) as constp,
            tc.tile_pool(name="sup", bufs=1) as supp,
            tc.tile_pool(name="x0s", bufs=3) as x0p,
            tc.tile_pool(name="x0T", bufs=1) as x0Tp,
            tc.tile_pool(name="wst", bufs=8) as wp,
            tc.tile_pool(name="vt", bufs=24) as vp,
            tc.tile_pool(name="ut", bufs=32) as up,
            tc.tile_pool(name="pg", bufs=8) as pgp,
            tc.tile_pool(name="stg", bufs=4) as stgp,
            tc.tile_pool(name="ps", bufs=8, space="PSUM") as psp,
        ):
            # ---- constants ----
            eyemm = constp.tile([128, 128], F32R)
            nc.sync.dma_start(eyemm[:], eye_d[0:128, 0:128].bitcast(F32R))
            bias_bc = constp.tile([128, D], F32)
            nc.sync.dma_start(
                bias_bc[:], bias_d.unsqueeze(0).broadcast_to([128, D]))

            # ---- phase B: x0 load + transpose (PE) ----
            # x0T[dt] holds x0^T rows d in [dt*128,(dt+1)*128), cols = bn
            x0T = []
            for dt in range(DT):
                t = x0Tp.tile([128, BN], F32R, name=f"x0T{dt}")
                x0T.append(t)
            for bt in range(NT):
                x0t = x0p.tile([128, D], F32R, name=f"x0t{bt}", tag="x0t")
                nc.sync.dma_start(
                    x0t[:], inp_d[bt * 128:(bt + 1) * 128, :].bitcast(F32R))
                for dt in range(DT):
                    pst = psp.tile([128, 128], F32R, name=f"pstr{bt}_{dt}",
                                   tag="ps")
                    nc.tensor.transpose(
                        pst[:], x0t[:, dt * 128:(dt + 1) * 128], eyemm[:])
                    nc.scalar.copy(x0T[dt][:, bt * 128:(bt + 1) * 128], pst[:])

            # ---- phase A: support matrix S^T (scaled) ----
            st = []
            dcols = []
            for t in range(JT):
                adjt = supp.tile([128, N], F32, name=f"adjt{t}", tag="adjt",
                                 bufs=2)
                nc.sync.dma_start(adjt[:], adj_d[t * 128:(t + 1) * 128, :])
                eyet = supp.tile([128, N], F32, name=f"eyet{t}", tag="eyet",
                                 bufs=2)
                nc.sync.dma_start(eyet[:], eye_d[t * 128:(t + 1) * 128, :])
                a = supp.tile([128, N], F32, name=f"a{t}")
                nc.vector.tensor_add(a[:], adjt[:], eyet[:])
                rs = supp.tile([128, 1], F32, name=f"rs{t}")
                nc.vector.tensor_reduce(rs[:], a[:], axis=AX.X, op=ALU.add)
                sq = supp.tile([128, 1], F32, name=f"sq{t}")
                nc.scalar.sqrt(sq[:], rs[:])
                dcol = supp.tile([128, 1], F32, name=f"dcol{t}")
                nc.vector.reciprocal(dcol[:], sq[:])
                nc.sync.dma_start(dscr.ap()[t * 128:(t + 1) * 128], dcol[:])
                dcols.append(dcol)
                st.append((a, None))
            dbc = constp.tile([128, N], F32)
            nc.sync.dma_start(
                dbc[:], dscr.ap().unsqueeze(0).broadcast_to([128, N]))
            st_t = []
            for t in range(JT):
                a, _ = st[t]
                s = supp.tile([128, N], F32R, name=f"st{t}")
                nc.vector.scalar_tensor_tensor(
                    s[:], a[:], dcols[t][:], dbc[:], ALU.mult, ALU.mult)
                st_t.append(s)

            # ---- per column-block: project then Horner ----
            for eb in range(EB):
                c0 = eb * WE
                # V weight combos for this column block
                v = [[None] * DT for _ in range(4)]
                for dt in range(DT):
                    w_raw = []
                    for m in range(4):
                        w = wp.tile([128, WE], F32, name=f"w{eb}_{dt}_{m}",
                                    tag="wt")
                        nc.sync.dma_start(
                            w[:],
                            wts_v[m, dt * 128:(dt + 1) * 128, c0:c0 + WE])
                        w_raw.append(w)
                    v0 = vp.tile([128, WE], F32R, name=f"v{eb}_{dt}_0",
                                 tag="vt")
                    nc.vector.tensor_sub(v0[:], w_raw[0][:], w_raw[2][:])
                    v1 = vp.tile([128, WE], F32R, name=f"v{eb}_{dt}_1",
                                 tag="vt")
                    nc.vector.scalar_tensor_tensor(
                        v1[:], w_raw[3][:], -3.0, w_raw[1][:],
                        ALU.mult, ALU.add)
                    v2 = vp.tile([128, WE], F32R, name=f"v{eb}_{dt}_2",
                                 tag="vt")
                    nc.vector.tensor_scalar_mul(v2[:], w_raw[2][:], 2.0)
                    v3 = vp.tile([128, WE], F32R, name=f"v{eb}_{dt}_3",
                                 tag="vt")
                    nc.vector.tensor_scalar_mul(v3[:], w_raw[3][:], 4.0)
                    v[0][dt], v[1][dt], v[2][dt], v[3][dt] = v0, v1, v2, v3

                for b in range(BL):
                    # projection for batch b: U_m[nt] = x0[b] @ V_m
                    u = [[None] * JT for _ in range(4)]
                    for nt in range(JT):
                        bt = b * JT + nt
                        pm = [psp.tile([128, WE], F32,
                                       name=f"pp{eb}_{bt}_{m}", tag="ps")
                              for m in range(4)]
                        for dt in range(DT):
                            lhs = x0T[dt][:, bt * 128:(bt + 1) * 128]
                            for m in range(4):
                                nc.tensor.matmul(
                                    pm[m][:], lhs, v[m][dt][:],
                                    start=(dt == 0), stop=(dt == DT - 1))
                        u0 = up.tile([128, WE], F32R,
                                     name=f"u{eb}_{bt}_0", tag="ut")
                        nc.vector.tensor_add(
                            u0[:], pm[0][:], bias_bc[:, c0:c0 + WE])
                        u[0][nt] = u0
                        for m in range(1, 4):
                            um = up.tile([128, WE], F32R,
                                         name=f"u{eb}_{bt}_{m}", tag="ut")
                            nc.scalar.copy(um[:], pm[m][:])
                            u[m][nt] = um

                    # Horner for batch b:
                    #   P2 = U2 + S P3 (fresh tiles: u[3] is still being
                    #                   read by later-traced matmuls)
                    #   P1 = U1 + S P2 (into u[3] tiles, now dead)
                    #   out = U0 + S P1 (staged + DMA)
                    src = u[3]
                    for step, (madd, dest) in enumerate(
                            [(2, "fresh"), (1, 3), (0, None)]):
                        new = [None] * JT
                        for nt in range(JT):
                            ph = psp.tile([128, WE], F32,
                                          name=f"phh{eb}_{b}_{step}_{nt}",
                                          tag="ps")
                            for jt in range(JT):
                                nc.tensor.matmul(
                                    ph[:],
                                    st_t[jt][:, nt * 128:(nt + 1) * 128],
                                    src[jt][:],
                                    start=(jt == 0), stop=(jt == JT - 1))
                            if dest == "fresh":
                                pgt = pgp.tile([128, WE], F32R,
                                               name=f"pg{eb}_{b}_{nt}",
                                               tag="pg")
                                nc.vector.tensor_add(
                                    pgt[:], ph[:], u[madd][nt][:])
                                new[nt] = pgt
                            elif dest is not None:
                                nc.vector.tensor_add(
                                    u[dest][nt][:], ph[:], u[madd][nt][:])
                                new[nt] = u[dest][nt]
                            else:
                                so = stgp.tile([128, WE], F32,
                                               name=f"so{eb}_{b}_{nt}",
                                               tag="outst")
                                nc.vector.tensor_add(
                                    so[:], ph[:], u[0][nt][:])
                                r0 = (b * JT + nt) * 128
                                nc.sync.dma_start(
                                    out_d[r0:r0 + 128, c0:c0 + WE], so[:])
                        src = new
    nc.compile()
    return nc


_CACHE = {}


def _get_program():
    if "nc" not in _CACHE:
        _CACHE["nc"] = _build_program()
    return _CACHE["nc"]


def kernel(inputs, adj, weights, biases):
    inputs = np.ascontiguousarray(inputs, dtype=np.float32)
    adj = np.ascontiguousarray(adj, dtype=np.float32)
    weights = np.ascontiguousarray(weights, dtype=np.float32)
    biases = np.ascontiguousarray(biases, dtype=np.float32)
    assert inputs.shape == (B, N, D)
    assert adj.shape == (N, N)
    assert weights.shape == (D * 4, D)
    assert biases.shape == (D,)

    nc = _get_program()
    eye = np.eye(N, dtype=np.float32)
    in_maps = []
    for c in range(N_CORES):
        in_maps.append({
            "inp": inputs[c * BL:(c + 1) * BL].reshape(BN, D),
            "adj": adj,
            "wts": weights,
            "bias": biases,
            "eye": eye,
        })
    res = run_bass_kernel_spmd(nc, in_maps, list(range(N_CORES)))
    out = np.concatenate(
        [res.results[c]["out"].reshape(BL, N, D) for c in range(N_CORES)],
        axis=0)
    return out


# revision 4
# speedup vs baseline: 1.0062x; 1.0062x over previous
"""DGCN diffusion-graph-conv kernel for 8 Trainium2 NeuronCores.

Math (per the reference):
    support S = D^-1/2 (adj+I)^T D^-1/2  with D = diag(rowsum(adj+I))
    x_m = T_m(S) x0  (Chebyshev recurrence, K=3 -> m=0..3)
    out = sum_m x_m @ W_m + bias

Implementation strategy (data-parallel over batch, 4 batches/core):
    Rewrite out = sum_m T_m(S) (x0 @ W_m) and fold the Chebyshev
    coefficients into the weights:
        V0 = W0 - W2, V1 = W1 - 3*W3, V2 = 2*W2, V3 = 4*W3
        U_m = x0 @ V_m   (projection; contracts feature dim d)
        out = U0 + S*(U1 + S*(U2 + S*U3))   (Horner; contracts node dim n)
    The projection's stationary operand is x0^T, which the host supplies
    directly (layout prep during sharding).  All matmuls run in fp32r
    (fp22 multiply / fp32 accumulate) at full PE rate.
"""

import numpy as np

import concourse.bacc as bacc
import concourse.tile as tile
import concourse.mybir as mybir
from concourse.bass_utils import run_bass_kernel_spmd

F32 = mybir.dt.float32
F32R = mybir.dt.float32r
AX = mybir.AxisListType
ALU = mybir.AluOpType

N_CORES = 8
B, N, D = 32, 512, 768
BL = B // N_CORES          # local batches per core = 4
BN = BL * N                # local rows = 2048
NT = BN // 128             # 16 row tiles
DT = D // 128              # 6 feature tiles
JT = N // 128              # 4 node tiles
WE = 256                   # output-column block width
EB = D // WE               # 3 column blocks


def _build_program():
    nc = bacc.Bacc("TRN2", target_bir_lowering=False, debug=False,
                   num_devices=N_CORES)
    # x0^T for this core: [d, (b n)]
    inpT_d = nc.dram_tensor("inpT", [D, BN], F32, kind="ExternalInput").ap()
    adj_d = nc.dram_tensor("adj", [N, N], F32, kind="ExternalInput").ap()
    wts_d = nc.dram_tensor("wts", [D * 4, D], F32, kind="ExternalInput").ap()
    bias_d = nc.dram_tensor("bias", [D], F32, kind="ExternalInput").ap()
    eye_d = nc.dram_tensor("eye", [N, N], F32, kind="ExternalInput").ap()
    out_d = nc.dram_tensor("out", [BN, D], F32, kind="ExternalOutput").ap()
    dscr = nc.dram_tensor("dscr", [N], F32)

    # weights viewed as [m, d, e] (reference row index is d*4+m)
    wts_v = wts_d.rearrange("(d m) e -> m d e", m=4)

    with tile.TileContext(nc) as tc:
        with (
            tc.tile_pool(name="const", bufs=1) as constp,
            tc.tile_pool(name="sup", bufs=1) as supp,
            tc.tile_pool(name="x0T", bufs=1) as x0Tp,
        ):
            # ---- x0^T load (straight DMA, chunked for early start) ----
            x0T = []
            for dt in range(DT):
                t = x0Tp.tile([128, BN], F32R, name=f"x0T{dt}")
                x0T.append(t)
            for ck in range(4):
                for dt in range(DT):
                    nc.sync.dma_start(
                        x0T[dt][:, ck * 512:(ck + 1) * 512],
                        inpT_d[dt * 128:(dt + 1) * 128,
                               ck * 512:(ck + 1) * 512].bitcast(F32R))

            bias_bc = constp.tile([128, D], F32)
            nc.sync.dma_start(
                bias_bc[:], bias_d.unsqueeze(0).broadcast_to([128, D]))

            # ---- support matrix S^T (scaled), scratch pool closed after ----
            st_t = []
            with tc.tile_pool(name="supscr", bufs=1) as scrp:
                avs, dcols = [], []
                for t in range(JT):
                    adjt = scrp.tile([128, N], F32, name=f"adjt{t}",
                                     tag="adjt", bufs=2)
                    nc.sync.dma_start(adjt[:], adj_d[t * 128:(t + 1) * 128, :])
                    eyet = scrp.tile([128, N], F32, name=f"eyet{t}",
                                     tag="eyet", bufs=2)
                    nc.sync.dma_start(eyet[:], eye_d[t * 128:(t + 1) * 128, :])
                    a = scrp.tile([128, N], F32, name=f"a{t}")
                    nc.vector.tensor_add(a[:], adjt[:], eyet[:])
                    rs = scrp.tile([128, 1], F32, name=f"rs{t}")
                    nc.vector.tensor_reduce(rs[:], a[:], axis=AX.X, op=ALU.add)
                    sq = scrp.tile([128, 1], F32, name=f"sq{t}")
                    nc.scalar.sqrt(sq[:], rs[:])
                    dcol = supp.tile([128, 1], F32, name=f"dcol{t}")
                    nc.vector.reciprocal(dcol[:], sq[:])
                    nc.sync.dma_start(
                        dscr.ap()[t * 128:(t + 1) * 128], dcol[:])
                    avs.append(a)
                    dcols.append(dcol)
                dbc = constp.tile([128, N], F32)
                nc.sync.dma_start(
                    dbc[:], dscr.ap().unsqueeze(0).broadcast_to([128, N]))
                for t in range(JT):
                    s = supp.tile([128, N], F32R, name=f"st{t}")
                    nc.vector.scalar_tensor_tensor(
                        s[:], avs[t][:], dcols[t][:], dbc[:],
                        ALU.mult, ALU.mult)
                    st_t.append(s)

            # ---- main loops: per column-block project then Horner ----
            with (
                tc.tile_pool(name="wst", bufs=8) as wp,
                tc.tile_pool(name="vt", bufs=48) as vp,
                tc.tile_pool(name="ut", bufs=32) as up,
                tc.tile_pool(name="pg", bufs=8) as pgp,
                tc.tile_pool(name="stg", bufs=4) as stgp,
                tc.tile_pool(name="ps", bufs=8, space="PSUM") as psp,
            ):
                def load_v(eb):
                    """DMA the W column block and build the V combos."""
                    c0 = eb * WE
                    v = [[None] * DT for _ in range(4)]
                    for dt in range(DT):
                        w_raw = []
                        for m in range(4):
                            w = wp.tile([128, WE], F32,
                                        name=f"w{eb}_{dt}_{m}", tag="wt")
                            nc.sync.dma_start(
                                w[:],
                                wts_v[m, dt * 128:(dt + 1) * 128,
                                      c0:c0 + WE])
                            w_raw.append(w)
                        v0 = vp.tile([128, WE], F32R, name=f"v{eb}_{dt}_0",
                                     tag="vt")
                        nc.vector.tensor_sub(v0[:], w_raw[0][:], w_raw[2][:])
                        v1 = vp.tile([128, WE], F32R, name=f"v{eb}_{dt}_1",
                                     tag="vt")
                        nc.vector.scalar_tensor_tensor(
                            v1[:], w_raw[3][:], -3.0, w_raw[1][:],
                            ALU.mult, ALU.add)
                        v2 = vp.tile([128, WE], F32R, name=f"v{eb}_{dt}_2",
                                     tag="vt")
                        nc.vector.tensor_scalar_mul(v2[:], w_raw[2][:], 2.0)
                        v3 = vp.tile([128, WE], F32R, name=f"v{eb}_{dt}_3",
                                     tag="vt")
                        nc.vector.tensor_scalar_mul(v3[:], w_raw[3][:], 4.0)
                        v[0][dt], v[1][dt] = v0, v1
                        v[2][dt], v[3][dt] = v2, v3
                    return v

                v_cur = load_v(0)
                for eb in range(EB):
                    c0 = eb * WE
                    v = v_cur
                    for b in range(BL):
                        # projection for batch b: U_m[nt] = x0[b] @ V_m
                        u = [[None] * JT for _ in range(4)]
                        for nt in range(JT):
                            bt = b * JT + nt
                            pm = [psp.tile([128, WE], F32,
                                           name=f"pp{eb}_{bt}_{m}", tag="ps")
                                  for m in range(4)]
                            for dt in range(DT):
                                lhs = x0T[dt][:, bt * 128:(bt + 1) * 128]
                                for m in range(4):
                                    nc.tensor.matmul(
                                        pm[m][:], lhs, v[m][dt][:],
                                        start=(dt == 0), stop=(dt == DT - 1))
                            u0 = up.tile([128, WE], F32R,
                                         name=f"u{eb}_{bt}_0", tag="ut")
                            nc.vector.tensor_add(
                                u0[:], pm[0][:], bias_bc[:, c0:c0 + WE])
                            u[0][nt] = u0
                            for m in range(1, 4):
                                um = up.tile([128, WE], F32R,
                                             name=f"u{eb}_{bt}_{m}", tag="ut")
                                nc.scalar.copy(um[:], pm[m][:])
                                u[m][nt] = um

                        if b == 0 and eb + 1 < EB:
                            # prefetch next column block's weights
                            v_cur = load_v(eb + 1)

                        # Horner for batch b:
                        #   P2 = U2 + S P3 (fresh tiles: u[3] is still being
                        #                   read by later-traced matmuls)
                        #   P1 = U1 + S P2 (into u[3] tiles, now dead)
                        #   out = U0 + S P1 (staged + DMA)
                        src = u[3]
                        for step, (madd, dest) in enumerate(
                                [(2, "fresh"), (1, 3), (0, None)]):
                            new = [None] * JT
                            for nt in range(JT):
                                ph = psp.tile([128, WE], F32,
                                              name=f"phh{eb}_{b}_{step}_{nt}",
                                              tag="ps")
                                for jt in range(JT):
                                    nc.tensor.matmul(
                                        ph[:],
                                        st_t[jt][:, nt * 128:(nt + 1) * 128],
                                        src[jt][:],
                                        start=(jt == 0), stop=(jt == JT - 1))
                                if dest == "fresh":
                                    pgt = pgp.tile([128, WE], F32R,
                                                   name=f"pg{eb}_{b}_{nt}",
                                                   tag="pg")
                                    nc.vector.tensor_add(
                                        pgt[:], ph[:], u[madd][nt][:])
                                    new[nt] = pgt
                                elif dest is not None:
                                    nc.vector.tensor_add(
                                        u[dest][nt][:], ph[:], u[madd][nt][:])
                                    new[nt] = u[dest][nt]
                                else:
                                    so = stgp.tile([128, WE], F32,
                                                   name=f"so{eb}_{b}_{nt}",
                                                   tag="outst")
                                    nc.vector.tensor_add(
                                        so[:], ph[:], u[0][nt][:])
                                    r0 = (b * JT + nt) * 128
                                    nc.sync.dma_start(
                                        out_d[r0:r0 + 128, c0:c0 + WE],
                                        so[:])
                            src = new
    nc.compile()
    return nc


_CACHE = {}


def _get_program():
    if "nc" not in _CACHE:
        _CACHE["nc"] = _build_program()
    return _CACHE["nc"]


def make_in_maps(inputs, adj, weights, biases):
    inputs = np.ascontiguousarray(inputs, dtype=np.float32)
    adj = np.ascontiguousarray(adj, dtype=np.float32)
    weights = np.ascontiguousarray(weights, dtype=np.float32)
    biases = np.ascontiguousarray(biases, dtype=np.float32)
    assert inputs.shape == (B, N, D)
    assert adj.shape == (N, N)
    assert weights.shape == (D * 4, D)
    assert biases.shape == (D,)
    eye = np.eye(N, dtype=np.float32)
    in_maps = []
    for c in range(N_CORES):
        x0T = np.ascontiguousarray(
            inputs[c * BL:(c + 1) * BL].reshape(BN, D).T)
        in_maps.append({
            "inpT": x0T,
            "adj": adj,
            "wts": weights,
            "bias": biases,
            "eye": eye,
        })
    return in_maps


def kernel(inputs, adj, weights, biases):
    nc = _get_program()
    in_maps = make_in_maps(inputs, adj, weights, biases)
    res = run_bass_kernel_spmd(nc, in_maps, list(range(N_CORES)))
    out = np.concatenate(
        [res.results[c]["out"].reshape(BL, N, D) for c in range(N_CORES)],
        axis=0)
    return out


# revision 8
# speedup vs baseline: 1.0627x; 1.0562x over previous
"""DGCN diffusion-graph-conv kernel for 8 Trainium2 NeuronCores.

Math (per the reference):
    support S = D^-1/2 (adj+I)^T D^-1/2  with D = diag(rowsum(adj+I))
    x_m = T_m(S) x0  (Chebyshev recurrence, K=3 -> m=0..3)
    out = sum_m x_m @ W_m + bias

Implementation strategy (data-parallel over batch, 4 batches/core):
    Rewrite out = sum_m T_m(S) (x0 @ W_m) and fold the Chebyshev
    coefficients into the weights:
        V0 = W0 - W2, V1 = W1 - 3*W3, V2 = 2*W2, V3 = 4*W3
        U_m = x0 @ V_m   (projection; contracts feature dim d)
        out = U0 + S*(U1 + S*(U2 + S*U3))   (Horner; contracts node dim n)
    The projection's stationary operand is x0^T, which the host supplies
    directly (layout prep during sharding).  All matmuls run in fp32r
    (fp22 multiply / fp32 accumulate) at full PE rate.
"""

import numpy as np

import concourse.bacc as bacc
import concourse.tile as tile
import concourse.mybir as mybir
from concourse.bass_utils import run_bass_kernel_spmd

F32 = mybir.dt.float32
F32R = mybir.dt.float32r
AX = mybir.AxisListType
ALU = mybir.AluOpType

N_CORES = 8
B, N, D = 32, 512, 768
BL = B // N_CORES          # local batches per core = 4
BN = BL * N                # local rows = 2048
NT = BN // 128             # 16 row tiles
DT = D // 128              # 6 feature tiles
JT = N // 128              # 4 node tiles
WE = 256                   # output-column block width
EB = D // WE               # 3 column blocks


def _build_program():
    nc = bacc.Bacc("TRN2", target_bir_lowering=False, debug=False,
                   num_devices=N_CORES)
    # x0^T for this core: [d, (b n)]
    inpT_d = nc.dram_tensor("inpT", [D, BN], F32, kind="ExternalInput").ap()
    adj_d = nc.dram_tensor("adj", [N, N], F32, kind="ExternalInput").ap()
    wts_d = nc.dram_tensor("wts", [D * 4, D], F32, kind="ExternalInput").ap()
    bias_d = nc.dram_tensor("bias", [D], F32, kind="ExternalInput").ap()
    eye_d = nc.dram_tensor("eye", [128, 128], F32, kind="ExternalInput").ap()
    out_d = nc.dram_tensor("out", [BN, D], F32, kind="ExternalOutput").ap()
    dscr = nc.dram_tensor("dscr", [N], F32)

    # weights viewed as [m, d, e] (reference row index is d*4+m)
    wts_v = wts_d.rearrange("(d m) e -> m d e", m=4)

    with tile.TileContext(nc) as tc:
        with (
            tc.tile_pool(name="const", bufs=1) as constp,
            tc.tile_pool(name="sup", bufs=1) as supp,
            tc.tile_pool(name="x0T", bufs=1) as x0Tp,
            tc.tile_pool(name="wst", bufs=8) as wp,
            tc.tile_pool(name="vt", bufs=48) as vp,
            tc.tile_pool(name="ut", bufs=32) as up,
            tc.tile_pool(name="pg", bufs=8) as pgp,
            tc.tile_pool(name="stg", bufs=4) as stgp,
            tc.tile_pool(name="ps", bufs=8, space="PSUM") as psp,
        ):
            def load_v(eb):
                """DMA the W column block and build the V combos."""
                c0 = eb * WE
                v = [[None] * DT for _ in range(4)]
                for dt in range(DT):
                    w_raw = []
                    for m in range(4):
                        w = wp.tile([128, WE], F32,
                                    name=f"w{eb}_{dt}_{m}", tag="wt")
                        nc.sync.dma_start(
                            w[:],
                            wts_v[m, dt * 128:(dt + 1) * 128, c0:c0 + WE])
                        w_raw.append(w)
                    v0 = vp.tile([128, WE], F32R, name=f"v{eb}_{dt}_0",
                                 tag="vt")
                    nc.vector.tensor_sub(v0[:], w_raw[0][:], w_raw[2][:])
                    v1 = vp.tile([128, WE], F32R, name=f"v{eb}_{dt}_1",
                                 tag="vt")
                    nc.vector.scalar_tensor_tensor(
                        v1[:], w_raw[3][:], -3.0, w_raw[1][:],
                        ALU.mult, ALU.add)
                    v2 = vp.tile([128, WE], F32R, name=f"v{eb}_{dt}_2",
                                 tag="vt")
                    nc.vector.tensor_scalar_mul(v2[:], w_raw[2][:], 2.0)
                    v3 = vp.tile([128, WE], F32R, name=f"v{eb}_{dt}_3",
                                 tag="vt")
                    nc.vector.tensor_scalar_mul(v3[:], w_raw[3][:], 4.0)
                    v[0][dt], v[1][dt] = v0, v1
                    v[2][dt], v[3][dt] = v2, v3
                return v

            # ---- DMA issue order: first-needed first ----
            # x0^T chunk 0 (row tiles bt=0..3), then eb0 weights, then the
            # rest of x0^T, then support/bias inputs.
            x0T = []
            for dt in range(DT):
                t = x0Tp.tile([128, BN], F32R, name=f"x0T{dt}")
                x0T.append(t)
            for dt in range(DT):
                nc.sync.dma_start(
                    x0T[dt][:, 0:512],
                    inpT_d[dt * 128:(dt + 1) * 128, 0:512].bitcast(F32R))

            v_cur = load_v(0)

            for ck in range(1, 4):
                for dt in range(DT):
                    nc.sync.dma_start(
                        x0T[dt][:, ck * 512:(ck + 1) * 512],
                        inpT_d[dt * 128:(dt + 1) * 128,
                               ck * 512:(ck + 1) * 512].bitcast(F32R))

            bias_bc = constp.tile([128, D], F32)
            nc.sync.dma_start(
                bias_bc[:], bias_d.unsqueeze(0).broadcast_to([128, D]))

            # ---- support matrix S^T = (adj+I) * d[j]d[i], built as
            #      adj*d[j]d[i] plus a diagonal d^2 fix-up ----
            eye128 = constp.tile([128, 128], F32)
            nc.sync.dma_start(eye128[:], eye_d[:])
            adjts, dcols, dsqs = [], [], []
            for t in range(JT):
                adjt = supp.tile([128, N], F32, name=f"adjt{t}")
                nc.sync.dma_start(adjt[:], adj_d[t * 128:(t + 1) * 128, :])
                rs = supp.tile([128, 1], F32, name=f"rs{t}", tag="rs",
                               bufs=2)
                nc.vector.tensor_reduce(rs[:], adjt[:], axis=AX.X, op=ALU.add)
                nc.vector.tensor_scalar_add(rs[:], rs[:], 1.0)
                sq = supp.tile([128, 1], F32, name=f"sq{t}", tag="sq",
                               bufs=2)
                nc.scalar.sqrt(sq[:], rs[:])
                dcol = supp.tile([128, 1], F32, name=f"dcol{t}")
                nc.vector.reciprocal(dcol[:], sq[:])
                dsq = supp.tile([128, 1], F32, name=f"dsq{t}")
                nc.vector.tensor_mul(dsq[:], dcol[:], dcol[:])
                nc.sync.dma_start(dscr.ap()[t * 128:(t + 1) * 128], dcol[:])
                adjts.append(adjt)
                dcols.append(dcol)
                dsqs.append(dsq)
            dbc = constp.tile([128, N], F32)
            nc.sync.dma_start(
                dbc[:], dscr.ap().unsqueeze(0).broadcast_to([128, N]))
            st_t = []
            for t in range(JT):
                s = supp.tile([128, N], F32R, name=f"st{t}")
                nc.vector.scalar_tensor_tensor(
                    s[:], adjts[t][:], dcols[t][:], dbc[:],
                    ALU.mult, ALU.mult)
                diagfix = supp.tile([128, 128], F32, name=f"dfix{t}",
                                    tag="dfix", bufs=2)
                nc.vector.tensor_scalar_mul(diagfix[:], eye128[:], dsqs[t][:])
                nc.vector.tensor_add(
                    s[:, t * 128:(t + 1) * 128],
                    s[:, t * 128:(t + 1) * 128], diagfix[:])
                st_t.append(s)

            # ---- main loops: per column-block project then Horner ----
            if True:
                for eb in range(EB):
                    c0 = eb * WE
                    v = v_cur
                    for b in range(BL):
                        # projection for batch b: U_m[nt] = x0[b] @ V_m
                        u = [[None] * JT for _ in range(4)]
                        for nt in range(JT):
                            bt = b * JT + nt
                            pm = [psp.tile([128, WE], F32,
                                           name=f"pp{eb}_{bt}_{m}", tag="ps")
                                  for m in range(4)]
                            for dt in range(DT):
                                lhs = x0T[dt][:, bt * 128:(bt + 1) * 128]
                                for m in range(4):
                                    nc.tensor.matmul(
                                        pm[m][:], lhs, v[m][dt][:],
                                        start=(dt == 0), stop=(dt == DT - 1))
                            u0 = up.tile([128, WE], F32R,
                                         name=f"u{eb}_{bt}_0", tag="ut")
                            nc.vector.tensor_add(
                                u0[:], pm[0][:], bias_bc[:, c0:c0 + WE])
                            u[0][nt] = u0
                            for m in range(1, 4):
                                um = up.tile([128, WE], F32R,
                                             name=f"u{eb}_{bt}_{m}", tag="ut")
                                nc.scalar.copy(um[:], pm[m][:])
                                u[m][nt] = um

                        if b == 0 and eb + 1 < EB:
                            # prefetch next column block's weights
                            v_cur = load_v(eb + 1)

                        # Horner for batch b:
                        #   P2 = U2 + S P3 (fresh tiles: u[3] is still being
                        #                   read by later-traced matmuls)
                        #   P1 = U1 + S P2 (into u[3] tiles, now dead)
                        #   out = U0 + S P1 (staged + DMA)
                        src = u[3]
                        for step, (madd, dest) in enumerate(
                                [(2, "fresh"), (1, 3), (0, None)]):
                            new = [None] * JT
                            for nt in range(JT):
                                ph = psp.tile([128, WE], F32,
                                              name=f"phh{eb}_{b}_{step}_{nt}",
                                              tag="ps")
                                for jt in range(JT):
                                    nc.tensor.matmul(
                                        ph[:],
                                        st_t[jt][:, nt * 128:(nt + 1) * 128],
                                        src[jt][:],
                                        start=(jt == 0), stop=(jt == JT - 1))
                                if dest == "fresh":
                                    pgt = pgp.tile([128, WE], F32R,
                                                   name=f"pg{eb}_{b}_{nt}",
                                                   tag="pg")
                                    nc.vector.tensor_add(
                                        pgt[:], ph[:], u[madd][nt][:])
                                    new[nt] = pgt
                                elif dest is not None:
                                    nc.vector.tensor_add(
                                        u[dest][nt][:], ph[:], u[madd][nt][:])
                                    new[nt] = u[dest][nt]
                                else:
                                    so = stgp.tile([128, WE], F32,
                                                   name=f"so{eb}_{b}_{nt}",
                                                   tag="outst")
                                    nc.vector.tensor_add(
                                        so[:], ph[:], u[0][nt][:])
                                    r0 = (b * JT + nt) * 128
                                    nc.sync.dma_start(
                                        out_d[r0:r0 + 128, c0:c0 + WE],
                                        so[:])
                            src = new
    nc.compile()
    return nc


_CACHE = {}


def _get_program():
    if "nc" not in _CACHE:
        _CACHE["nc"] = _build_program()
    return _CACHE["nc"]


def make_in_maps(inputs, adj, weights, biases):
    inputs = np.ascontiguousarray(inputs, dtype=np.float32)
    adj = np.ascontiguousarray(adj, dtype=np.float32)
    weights = np.ascontiguousarray(weights, dtype=np.float32)
    biases = np.ascontiguousarray(biases, dtype=np.float32)
    assert inputs.shape == (B, N, D)
    assert adj.shape == (N, N)
    assert weights.shape == (D * 4, D)
    assert biases.shape == (D,)
    eye = np.eye(128, dtype=np.float32)
    in_maps = []
    for c in range(N_CORES):
        x0T = np.ascontiguousarray(
            inputs[c * BL:(c + 1) * BL].reshape(BN, D).T)
        in_maps.append({
            "inpT": x0T,
            "adj": adj,
            "wts": weights,
            "bias": biases,
            "eye": eye,
        })
    return in_maps


def kernel(inputs, adj, weights, biases):
    nc = _get_program()
    in_maps = make_in_maps(inputs, adj, weights, biases)
    res = run_bass_kernel_spmd(nc, in_maps, list(range(N_CORES)))
    out = np.concatenate(
        [res.results[c]["out"].reshape(BL, N, D) for c in range(N_CORES)],
        axis=0)
    return out


# revision 22
# speedup vs baseline: 1.1879x; 1.1177x over previous
"""DGCN diffusion-graph-conv kernel for 8 Trainium2 NeuronCores.

Math (per the reference):
    support S = D^-1/2 (adj+I)^T D^-1/2  with D = diag(rowsum(adj+I))
    x_m = T_m(S) x0  (Chebyshev recurrence, K=3 -> m=0..3)
    out = sum_m x_m @ W_m + bias

Implementation strategy (data-parallel over batch, 4 batches/core):
    Rewrite out = sum_m T_m(S) (x0 @ W_m) and fold the Chebyshev
    coefficients into the weights:
        V0 = W0 - W2, V1 = W1 - 3*W3, V2 = 2*W2, V3 = 4*W3
        U_m = x0 @ V_m   (projection; contracts feature dim d)
        out = U0 + S*(U1 + S*(U2 + S*U3))   (Horner; contracts node dim n)
    The projection's stationary operand is x0^T, which the host supplies
    directly (layout prep during sharding).  All matmuls run in fp32r
    (fp22 multiply / fp32 accumulate) at full PE rate.
"""

import numpy as np

import concourse.bacc as bacc
import concourse.tile as tile
import concourse.mybir as mybir
from concourse.bass_utils import run_bass_kernel_spmd

F32 = mybir.dt.float32
F32R = mybir.dt.float32r
AX = mybir.AxisListType
ALU = mybir.AluOpType

N_CORES = 8
B, N, D = 32, 512, 768
BL = B // N_CORES          # local batches per core = 4
BN = BL * N                # local rows = 2048
NT = BN // 128             # 16 row tiles
DT = D // 128              # 6 feature tiles
JT = N // 128              # 4 node tiles
WE = 256                   # output-column block width
EB = D // WE               # 3 column blocks


def _build_program():
    nc = bacc.Bacc("TRN2", target_bir_lowering=False, debug=False,
                   num_devices=N_CORES)
    # x0^T for this core: [d, (b n)]
    inpT_d = nc.dram_tensor("inpT", [D, BN], F32, kind="ExternalInput").ap()
    adj_d = nc.dram_tensor("adj", [N, N], F32, kind="ExternalInput").ap()
    wts_d = nc.dram_tensor("wts", [D * 4, D], F32, kind="ExternalInput").ap()
    bias_d = nc.dram_tensor("bias", [D], F32, kind="ExternalInput").ap()
    eye_d = nc.dram_tensor("eye", [128, 128], F32, kind="ExternalInput").ap()
    out_d = nc.dram_tensor("out", [BN, D], F32, kind="ExternalOutput").ap()
    dscr = nc.dram_tensor("dscr", [N], F32)

    # weights viewed as [m, d, e] (reference row index is d*4+m)
    wts_v = wts_d.rearrange("(d m) e -> m d e", m=4)

    with tile.TileContext(nc) as tc:
        with (
            tc.tile_pool(name="const", bufs=1) as constp,
            tc.tile_pool(name="sup", bufs=1) as supp,
            tc.tile_pool(name="x0T", bufs=1) as x0Tp,
            tc.tile_pool(name="wst", bufs=8) as wp,
            tc.tile_pool(name="vt", bufs=48) as vp,
            tc.tile_pool(name="ut", bufs=32) as up,
            tc.tile_pool(name="pg", bufs=8) as pgp,
            tc.tile_pool(name="stg", bufs=4) as stgp,
            tc.tile_pool(name="ps", bufs=8, space="PSUM") as psp,
        ):
            def load_v(eb, dts=None, v=None):
                """DMA the W column block and build the V combos."""
                c0 = eb * WE
                if v is None:
                    v = [[None] * DT for _ in range(4)]
                for dt in (dts if dts is not None else range(DT)):
                    w_raw = []
                    for m in range(4):
                        w = wp.tile([128, WE], F32,
                                    name=f"w{eb}_{dt}_{m}", tag="wt")
                        nc.sync.dma_start(
                            w[:],
                            wts_v[m, dt * 128:(dt + 1) * 128, c0:c0 + WE])
                        w_raw.append(w[:])
                    v0 = vp.tile([128, WE], F32R, name=f"v{eb}_{dt}_0",
                                 tag="vt")
                    nc.vector.tensor_sub(v0[:], w_raw[0], w_raw[2])
                    v1 = vp.tile([128, WE], F32R, name=f"v{eb}_{dt}_1",
                                 tag="vt")
                    nc.vector.scalar_tensor_tensor(
                        v1[:], w_raw[3], -3.0, w_raw[1],
                        ALU.mult, ALU.add)
                    v2 = vp.tile([128, WE], F32R, name=f"v{eb}_{dt}_2",
                                 tag="vt")
                    nc.vector.tensor_scalar_mul(v2[:], w_raw[2], 2.0)
                    v3 = vp.tile([128, WE], F32R, name=f"v{eb}_{dt}_3",
                                 tag="vt")
                    nc.vector.tensor_scalar_mul(v3[:], w_raw[3], 4.0)
                    v[0][dt], v[1][dt] = v0, v1
                    v[2][dt], v[3][dt] = v2, v3
                return v

            eye128 = constp.tile([128, 128], F32)
            nc.gpsimd.dma_start(eye128[:], eye_d[:])

            # ---- DMA issue order: first-needed first ----
            # x0^T chunk 0 (row tiles bt=0..3), then eb0 weights, then the
            # rest of x0^T, then support/bias inputs.
            x0T = []
            for dt in range(DT):
                t = x0Tp.tile([128, BN], F32R, name=f"x0T{dt}")
                x0T.append(t)
            for dt in range(DT):
                nc.sync.dma_start(
                    x0T[dt][:, 0:512],
                    inpT_d[dt * 128:(dt + 1) * 128, 0:512].bitcast(F32R))

            adjts = []
            for t in range(JT):
                adjt = supp.tile([128, N], F32, name=f"adjt{t}")
                nc.sync.dma_start(adjt[:], adj_d[t * 128:(t + 1) * 128, :])
                adjts.append(adjt)

            v_cur = load_v(0)

            for ck in range(1, 4):
                for dt in range(DT):
                    nc.sync.dma_start(
                        x0T[dt][:, ck * 512:(ck + 1) * 512],
                        inpT_d[dt * 128:(dt + 1) * 128,
                               ck * 512:(ck + 1) * 512].bitcast(F32R))

            bias_bc = constp.tile([128, D], F32)
            nc.gpsimd.dma_start(
                bias_bc[:], bias_d.unsqueeze(0).broadcast_to([128, D]))

            # ---- support matrix S^T = (adj+I) * d[j]d[i], built as
            #      adj*d[j]d[i] plus a diagonal d^2 fix-up ----
            dcols, dsqs = [], []
            for t in range(JT):
                adjt = adjts[t]
                rs = supp.tile([128, 1], F32, name=f"rs{t}", tag="rs",
                               bufs=2)
                nc.vector.tensor_reduce(rs[:], adjt[:], axis=AX.X, op=ALU.add)
                nc.vector.tensor_scalar_add(rs[:], rs[:], 1.0)
                sq = supp.tile([128, 1], F32, name=f"sq{t}", tag="sq",
                               bufs=2)
                nc.scalar.sqrt(sq[:], rs[:])
                dcol = supp.tile([128, 1], F32, name=f"dcol{t}")
                nc.vector.reciprocal(dcol[:], sq[:])
                dsq = supp.tile([128, 1], F32, name=f"dsq{t}")
                nc.vector.tensor_mul(dsq[:], dcol[:], dcol[:])
                nc.gpsimd.dma_start(dscr.ap()[t * 128:(t + 1) * 128],
                                    dcol[:])
                dcols.append(dcol)
                dsqs.append(dsq)
            dbc = constp.tile([128, N], F32)
            nc.gpsimd.dma_start(
                dbc[:], dscr.ap().unsqueeze(0).broadcast_to([128, N]))
            st_t = []
            for t in range(JT):
                s = supp.tile([128, N], F32R, name=f"st{t}")
                nc.vector.scalar_tensor_tensor(
                    s[:], adjts[t][:], dcols[t][:], dbc[:],
                    ALU.mult, ALU.mult)
                diagfix = supp.tile([128, 128], F32, name=f"dfix{t}",
                                    tag="dfix", bufs=2)
                nc.vector.tensor_scalar_mul(diagfix[:], eye128[:], dsqs[t][:])
                nc.vector.tensor_add(
                    s[:, t * 128:(t + 1) * 128],
                    s[:, t * 128:(t + 1) * 128], diagfix[:])
                st_t.append(s)

            # ---- main loops: per column-block project then Horner ----
            if True:
                for eb in range(EB):
                    c0 = eb * WE
                    v = v_cur
                    for b in range(BL):
                        # projection for batch b: U_m[nt] = x0[b] @ V_m
                        u = [[None] * JT for _ in range(4)]
                        for nt in range(JT):
                            bt = b * JT + nt
                            for m in range(4):
                                pmt = psp.tile([128, WE], F32,
                                               name=f"pp{eb}_{bt}_{m}",
                                               tag="ps")
                                for dt in range(DT):
                                    lhs = x0T[dt][:, bt * 128:(bt + 1) * 128]
                                    nc.tensor.matmul(
                                        pmt[:], lhs, v[m][dt][:],
                                        start=(dt == 0), stop=(dt == DT - 1))
                                um = up.tile([128, WE], F32R,
                                             name=f"u{eb}_{bt}_{m}", tag="ut")
                                if m == 0:
                                    nc.vector.tensor_add(
                                        um[:], pmt[:], bias_bc[:, c0:c0 + WE])
                                else:
                                    nc.scalar.copy(um[:], pmt[:])
                                u[m][nt] = um

                        if b == 0 and eb + 1 < EB:
                            # prefetch next column block's weights
                            v_cur = load_v(eb + 1)

                        # Horner for batch b:
                        #   P2 = U2 + S P3 (fresh tiles: u[3] is still being
                        #                   read by later-traced matmuls)
                        #   P1 = U1 + S P2 (into u[3] tiles, now dead)
                        #   out = U0 + S P1 (staged + DMA)
                        src = u[3]
                        for step, (madd, dest) in enumerate(
                                [(2, "fresh"), (1, 3), (0, None)]):
                            new = [None] * JT
                            for nt in range(JT):
                                ph = psp.tile([128, WE], F32,
                                              name=f"phh{eb}_{b}_{step}_{nt}",
                                              tag="ps")
                                for jt in range(JT):
                                    nc.tensor.matmul(
                                        ph[:],
                                        st_t[jt][:, nt * 128:(nt + 1) * 128],
                                        src[jt][:],
                                        start=(jt == 0), stop=(jt == JT - 1))
                                if dest == "fresh":
                                    pgt = pgp.tile([128, WE], F32R,
                                                   name=f"pg{eb}_{b}_{nt}",
                                                   tag="pg")
                                    nc.vector.tensor_add(
                                        pgt[:], ph[:], u[madd][nt][:])
                                    new[nt] = pgt
                                elif dest is not None:
                                    nc.vector.tensor_add(
                                        u[dest][nt][:], ph[:], u[madd][nt][:])
                                    new[nt] = u[dest][nt]
                                else:
                                    so = stgp.tile([128, WE], F32,
                                                   name=f"so{eb}_{b}_{nt}",
                                                   tag="outst")
                                    nc.vector.tensor_add(
                                        so[:], ph[:], u[0][nt][:])
                                    r0 = (b * JT + nt) * 128
                                    nc.sync.dma_start(
                                        out_d[r0:r0 + 128, c0:c0 + WE],
                                        so[:])
                            src = new
    nc.compile()
    return nc


_CACHE = {}


def _get_program():
    if "nc" not in _CACHE:
        _CACHE["nc"] = _build_program()
    return _CACHE["nc"]


def make_in_maps(inputs, adj, weights, biases):
    inputs = np.ascontiguousarray(inputs, dtype=np.float32)
    adj = np.ascontiguousarray(adj, dtype=np.float32)
    weights = np.ascontiguousarray(weights, dtype=np.float32)
    biases = np.ascontiguousarray(biases, dtype=np.float32)
    assert inputs.shape == (B, N, D)
    assert adj.shape == (N, N)
    assert weights.shape == (D * 4, D)
    assert biases.shape == (D,)
    eye = np.eye(128, dtype=np.float32)
    in_maps = []
    for c in range(N_CORES):
        x0T = np.ascontiguousarray(
            inputs[c * BL:(c + 1) * BL].reshape(BN, D).T)
        in_maps.append({
            "inpT": x0T,
            "adj": adj,
            "wts": weights,
            "bias": biases,
            "eye": eye,
        })
    return in_maps


def kernel(inputs, adj, weights, biases):
    nc = _get_program()
    in_maps = make_in_maps(inputs, adj, weights, biases)
    res = run_bass_kernel_spmd(nc, in_maps, list(range(N_CORES)))
    out = np.concatenate(
        [res.results[c]["out"].reshape(BL, N, D) for c in range(N_CORES)],
        axis=0)
    return out


# revision 23
# speedup vs baseline: 1.2007x; 1.0108x over previous
"""DGCN diffusion-graph-conv kernel for 8 Trainium2 NeuronCores.

Math (per the reference):
    support S = D^-1/2 (adj+I)^T D^-1/2  with D = diag(rowsum(adj+I))
    x_m = T_m(S) x0  (Chebyshev recurrence, K=3 -> m=0..3)
    out = sum_m x_m @ W_m + bias

Implementation strategy (data-parallel over batch, 4 batches/core):
    Rewrite out = sum_m T_m(S) (x0 @ W_m) and fold the Chebyshev
    coefficients into the weights:
        V0 = W0 - W2, V1 = W1 - 3*W3, V2 = 2*W2, V3 = 4*W3
        U_m = x0 @ V_m   (projection; contracts feature dim d)
        out = U0 + S*(U1 + S*(U2 + S*U3))   (Horner; contracts node dim n)
    The projection's stationary operand is x0^T, which the host supplies
    directly (layout prep during sharding).  All matmuls run in fp32r
    (fp22 multiply / fp32 accumulate) at full PE rate.
"""

import numpy as np

import concourse.bacc as bacc
import concourse.tile as tile
import concourse.mybir as mybir
from concourse.bass_utils import run_bass_kernel_spmd

F32 = mybir.dt.float32
F32R = mybir.dt.float32r
AX = mybir.AxisListType
ALU = mybir.AluOpType

N_CORES = 8
B, N, D = 32, 512, 768
BL = B // N_CORES          # local batches per core = 4
BN = BL * N                # local rows = 2048
NT = BN // 128             # 16 row tiles
DT = D // 128              # 6 feature tiles
JT = N // 128              # 4 node tiles
WE = 256                   # output-column block width
EB = D // WE               # 3 column blocks


def _build_program():
    nc = bacc.Bacc("TRN2", target_bir_lowering=False, debug=False,
                   num_devices=N_CORES)
    # x0^T for this core: [d, (b n)]
    inpT_d = nc.dram_tensor("inpT", [D, BN], F32, kind="ExternalInput").ap()
    adj_d = nc.dram_tensor("adj", [N, N], F32, kind="ExternalInput").ap()
    wts_d = nc.dram_tensor("wts", [D * 4, D], F32, kind="ExternalInput").ap()
    bias_d = nc.dram_tensor("bias", [D], F32, kind="ExternalInput").ap()
    eye_d = nc.dram_tensor("eye", [128, 128], F32, kind="ExternalInput").ap()
    out_d = nc.dram_tensor("out", [BN, D], F32, kind="ExternalOutput").ap()
    dscr = nc.dram_tensor("dscr", [N], F32)

    # weights viewed as [m, d, e] (reference row index is d*4+m)
    wts_v = wts_d.rearrange("(d m) e -> m d e", m=4)

    with tile.TileContext(nc) as tc:
        with (
            tc.tile_pool(name="const", bufs=1) as constp,
            tc.tile_pool(name="sup", bufs=1) as supp,
            tc.tile_pool(name="x0T", bufs=1) as x0Tp,
            tc.tile_pool(name="wst", bufs=24) as wp,
            tc.tile_pool(name="vt", bufs=48) as vp,
            tc.tile_pool(name="ut", bufs=32) as up,
            tc.tile_pool(name="pg", bufs=8) as pgp,
            tc.tile_pool(name="stg", bufs=4) as stgp,
            tc.tile_pool(name="ps", bufs=8, space="PSUM") as psp,
        ):
            def load_v(eb, dts=None, v=None):
                """DMA the W column block and build the V combos."""
                c0 = eb * WE
                if v is None:
                    v = [[None] * DT for _ in range(4)]
                for dt in (dts if dts is not None else range(DT)):
                    w_raw = []
                    for m in range(4):
                        w = wp.tile([128, WE], F32,
                                    name=f"w{eb}_{dt}_{m}", tag="wt")
                        nc.sync.dma_start(
                            w[:],
                            wts_v[m, dt * 128:(dt + 1) * 128, c0:c0 + WE])
                        w_raw.append(w[:])
                    v0 = vp.tile([128, WE], F32R, name=f"v{eb}_{dt}_0",
                                 tag="vt")
                    nc.vector.tensor_sub(v0[:], w_raw[0], w_raw[2])
                    v1 = vp.tile([128, WE], F32R, name=f"v{eb}_{dt}_1",
                                 tag="vt")
                    nc.vector.scalar_tensor_tensor(
                        v1[:], w_raw[3], -3.0, w_raw[1],
                        ALU.mult, ALU.add)
                    v2 = vp.tile([128, WE], F32R, name=f"v{eb}_{dt}_2",
                                 tag="vt")
                    nc.vector.tensor_scalar_mul(v2[:], w_raw[2], 2.0)
                    v3 = vp.tile([128, WE], F32R, name=f"v{eb}_{dt}_3",
                                 tag="vt")
                    nc.vector.tensor_scalar_mul(v3[:], w_raw[3], 4.0)
                    v[0][dt], v[1][dt] = v0, v1
                    v[2][dt], v[3][dt] = v2, v3
                return v

            eye128 = constp.tile([128, 128], F32)
            nc.gpsimd.dma_start(eye128[:], eye_d[:])

            # ---- DMA issue order: first-needed first ----
            # x0^T chunk 0 (row tiles bt=0..3), then eb0 weights, then the
            # rest of x0^T, then support/bias inputs.
            x0T = []
            for dt in range(DT):
                t = x0Tp.tile([128, BN], F32R, name=f"x0T{dt}")
                x0T.append(t)
            for dt in range(DT):
                nc.sync.dma_start(
                    x0T[dt][:, 0:512],
                    inpT_d[dt * 128:(dt + 1) * 128, 0:512].bitcast(F32R))

            adjts = []
            for t in range(JT):
                adjt = supp.tile([128, N], F32, name=f"adjt{t}")
                nc.sync.dma_start(adjt[:], adj_d[t * 128:(t + 1) * 128, :])
                adjts.append(adjt)

            v_cur = load_v(0)

            for ck in range(1, 4):
                for dt in range(DT):
                    nc.sync.dma_start(
                        x0T[dt][:, ck * 512:(ck + 1) * 512],
                        inpT_d[dt * 128:(dt + 1) * 128,
                               ck * 512:(ck + 1) * 512].bitcast(F32R))

            bias_bc = constp.tile([128, D], F32)
            nc.gpsimd.dma_start(
                bias_bc[:], bias_d.unsqueeze(0).broadcast_to([128, D]))

            # ---- support matrix S^T = (adj+I) * d[j]d[i], built as
            #      adj*d[j]d[i] plus a diagonal d^2 fix-up ----
            dcols, dsqs = [], []
            for t in range(JT):
                adjt = adjts[t]
                rs = supp.tile([128, 1], F32, name=f"rs{t}", tag="rs",
                               bufs=2)
                nc.vector.tensor_reduce(rs[:], adjt[:], axis=AX.X, op=ALU.add)
                nc.vector.tensor_scalar_add(rs[:], rs[:], 1.0)
                sq = supp.tile([128, 1], F32, name=f"sq{t}", tag="sq",
                               bufs=2)
                nc.scalar.sqrt(sq[:], rs[:])
                dcol = supp.tile([128, 1], F32, name=f"dcol{t}")
                nc.vector.reciprocal(dcol[:], sq[:])
                dsq = supp.tile([128, 1], F32, name=f"dsq{t}")
                nc.vector.tensor_mul(dsq[:], dcol[:], dcol[:])
                nc.gpsimd.dma_start(dscr.ap()[t * 128:(t + 1) * 128],
                                    dcol[:])
                dcols.append(dcol)
                dsqs.append(dsq)
            dbc = constp.tile([128, N], F32)
            nc.gpsimd.dma_start(
                dbc[:], dscr.ap().unsqueeze(0).broadcast_to([128, N]))
            st_t = []
            for t in range(JT):
                s = supp.tile([128, N], F32R, name=f"st{t}")
                nc.vector.scalar_tensor_tensor(
                    s[:], adjts[t][:], dcols[t][:], dbc[:],
                    ALU.mult, ALU.mult)
                diagfix = supp.tile([128, 128], F32, name=f"dfix{t}",
                                    tag="dfix", bufs=2)
                nc.vector.tensor_scalar_mul(diagfix[:], eye128[:], dsqs[t][:])
                nc.vector.tensor_add(
                    s[:, t * 128:(t + 1) * 128],
                    s[:, t * 128:(t + 1) * 128], diagfix[:])
                st_t.append(s)

            # ---- main loops: per column-block project then Horner ----
            if True:
                for eb in range(EB):
                    c0 = eb * WE
                    v = v_cur
                    for b in range(BL):
                        # projection for batch b: U_m[nt] = x0[b] @ V_m
                        u = [[None] * JT for _ in range(4)]
                        for nt in range(JT):
                            bt = b * JT + nt
                            for m in range(4):
                                pmt = psp.tile([128, WE], F32,
                                               name=f"pp{eb}_{bt}_{m}",
                                               tag="ps")
                                for dt in range(DT):
                                    lhs = x0T[dt][:, bt * 128:(bt + 1) * 128]
                                    nc.tensor.matmul(
                                        pmt[:], lhs, v[m][dt][:],
                                        start=(dt == 0), stop=(dt == DT - 1))
                                um = up.tile([128, WE], F32R,
                                             name=f"u{eb}_{bt}_{m}", tag="ut")
                                if m == 0:
                                    nc.vector.tensor_add(
                                        um[:], pmt[:], bias_bc[:, c0:c0 + WE])
                                else:
                                    nc.scalar.copy(um[:], pmt[:])
                                u[m][nt] = um

                        if b == 0 and eb + 1 < EB:
                            # prefetch next column block's weights
                            v_cur = load_v(eb + 1)

                        # Horner for batch b:
                        #   P2 = U2 + S P3 (fresh tiles: u[3] is still being
                        #                   read by later-traced matmuls)
                        #   P1 = U1 + S P2 (into u[3] tiles, now dead)
                        #   out = U0 + S P1 (staged + DMA)
                        src = u[3]
                        for step, (madd, dest) in enumerate(
                                [(2, "fresh"), (1, 3), (0, None)]):
                            new = [None] * JT
                            for nt in range(JT):
                                ph = psp.tile([128, WE], F32,
                                              name=f"phh{eb}_{b}_{step}_{nt}",
                                              tag="ps")
                                for jt in range(JT):
                                    nc.tensor.matmul(
                                        ph[:],
                                        st_t[jt][:, nt * 128:(nt + 1) * 128],
                                        src[jt][:],
                                        start=(jt == 0), stop=(jt == JT - 1))
                                if dest == "fresh":
                                    pgt = pgp.tile([128, WE], F32R,
                                                   name=f"pg{eb}_{b}_{nt}",
                                                   tag="pg")
                                    nc.vector.tensor_add(
                                        pgt[:], ph[:], u[madd][nt][:])
                                    new[nt] = pgt
                                elif dest is not None:
                                    nc.vector.tensor_add(
                                        u[dest][nt][:], ph[:], u[madd][nt][:])
                                    new[nt] = u[dest][nt]
                                else:
                                    so = stgp.tile([128, WE], F32,
                                                   name=f"so{eb}_{b}_{nt}",
                                                   tag="outst")
                                    nc.vector.tensor_add(
                                        so[:], ph[:], u[0][nt][:])
                                    r0 = (b * JT + nt) * 128
                                    nc.sync.dma_start(
                                        out_d[r0:r0 + 128, c0:c0 + WE],
                                        so[:])
                            src = new
    nc.compile()
    return nc


_CACHE = {}


def _get_program():
    if "nc" not in _CACHE:
        _CACHE["nc"] = _build_program()
    return _CACHE["nc"]


def make_in_maps(inputs, adj, weights, biases):
    inputs = np.ascontiguousarray(inputs, dtype=np.float32)
    adj = np.ascontiguousarray(adj, dtype=np.float32)
    weights = np.ascontiguousarray(weights, dtype=np.float32)
    biases = np.ascontiguousarray(biases, dtype=np.float32)
    assert inputs.shape == (B, N, D)
    assert adj.shape == (N, N)
    assert weights.shape == (D * 4, D)
    assert biases.shape == (D,)
    eye = np.eye(128, dtype=np.float32)
    in_maps = []
    for c in range(N_CORES):
        x0T = np.ascontiguousarray(
            inputs[c * BL:(c + 1) * BL].reshape(BN, D).T)
        in_maps.append({
            "inpT": x0T,
            "adj": adj,
            "wts": weights,
            "bias": biases,
            "eye": eye,
        })
    return in_maps


def kernel(inputs, adj, weights, biases):
    nc = _get_program()
    in_maps = make_in_maps(inputs, adj, weights, biases)
    res = run_bass_kernel_spmd(nc, in_maps, list(range(N_CORES)))
    out = np.concatenate(
        [res.results[c]["out"].reshape(BL, N, D) for c in range(N_CORES)],
        axis=0)
    return out
